# revision 1
# baseline (speedup 1.0000x reference)
"""Fused transformer block (QKV -> diag-zeroed attention -> FFN -> LayerNorm)
for Trainium2, head-sharded over 8 NeuronCores with an AllToAll.

Sharding: core c owns head pair c (heads 2c, 2c+1) for attention over ALL
tokens of both batches -- no redundant K/V work. The attention outputs are
exchanged with a single AllToAll so core c then owns token block c
(batch c//4, queries [(c%4)*512, ...+512)) with the full model dim, and runs
the FFN + LayerNorm for those tokens.

Attention math (diagonal zeroed *after* softmax, per reference):
  eT[k, q] = exp(score[k, q] / 32)             (scores are small: no max-sub)
  num[d, q], denom[q] = (V | ones)^T @ eT      (ones-augmented V matmul)
  out[d, q] = (num[d,q] - eT[q,q] * V[q,d]) / denom[q]

Precision: projections and attention internals in bf16 (errors there are
attenuated by softmax averaging); the FFN path (attention out, W1, h1, W2)
in fp32 with float32r matmuls, since LayerNorm renormalizes the small FFN
signal and any relative error there lands directly on the output.
"""

import os
import numpy as np
import ml_dtypes
from contextlib import ExitStack

import concourse.bass as bass
import concourse.mybir as mybir
import concourse.tile as tile
from concourse import bacc
from concourse.bass_utils import run_bass_kernel_spmd

AF = mybir.ActivationFunctionType
ALU = mybir.AluOpType
BF16 = mybir.dt.bfloat16
F32 = mybir.dt.float32
F32R = mybir.dt.float32r

N_CORES = 8
B, S, D, H, HD, F = 2, 2048, 1024, 16, 64, 4096
QB = 512          # tokens per core after the exchange
CT = D // 128     # 8 contraction tiles over D
TT = S // 512     # 4 token 512-blocks per batch
NKT = S // 128    # 16 key 128-tiles per batch
FT = F // 128     # 32 f 128-tiles
INV_SQRT_D = 1.0 / 32.0
LN_EPS = 1e-5
VW = HD + 1       # 65: V columns per head incl. the ones column

_NC = None


def _r(ap, pattern, **kw):
    return ap.rearrange(pattern, **kw)


def _emit(tc, nc, io):
    KPH = os.environ.get("BASS_KERNEL_BISECT", "full")
    ts = bass.ts

    def bisect_out(pool, tiles, n=4):
        for i in range(n):
            st = pool.tile([128, 512], F32, tag="bis", name="bis")
            nc.vector.tensor_copy(st[:], tiles[i][:, 0:512])
            nc.sync.dma_start(io["y"][ts(i, 128), 0:512], st[:])
    with ExitStack() as ctx:
        # ---------------- constants ----------------------------------------
        cpool = ctx.enter_context(tc.tile_pool(name="consts", bufs=1))
        eye_sb = cpool.tile([128, 128], BF16)
        nc.sync.dma_start(eye_sb[:], io["eye"][:])
        bq_sb = cpool.tile([128, 1], F32)
        nc.sync.dma_start(bq_sb[:], io["bq_hp"][:])
        bk_sb = cpool.tile([128, 1], F32)
        nc.sync.dma_start(bk_sb[:], io["bk_hp"][:])
        bv_sb = cpool.tile([128, 1], F32)
        nc.sync.dma_start(bv_sb[:], io["bv_hp"][:])
        bvbc_sb = cpool.tile([128, 128], F32)
        nc.sync.dma_start(bvbc_sb[:], io["bv_bc2"][:])
        b1_sb = cpool.tile([128, FT], F32)
        nc.sync.dma_start(b1_sb[:], io["b1_r"][:])
        eps_sb = cpool.tile([128, 1], F32)
        nc.vector.memset(eps_sb[:], LN_EPS)

        # outt: token-major attention output after the exchange (fp32)
        outtp = ctx.enter_context(tc.tile_pool(name="outtp", bufs=CT))
        outt_sb = [outtp.tile([128, QB], F32R, tag="outt", name="outt")
                   for _ in range(CT)]

        a2a_in = nc.dram_tensor("a2a_in", [N_CORES, 128, QB], BF16).ap()
        a2a_out = nc.dram_tensor("a2a_out", [N_CORES, 128, QB], BF16).ap()

        with ExitStack() as actx:
            ktp = actx.enter_context(tc.tile_pool(name="ktp", bufs=2))
            qtp = actx.enter_context(tc.tile_pool(name="qtp", bufs=2))
            vtp = actx.enter_context(tc.tile_pool(name="vtp", bufs=2))
            vp = actx.enter_context(tc.tile_pool(name="vp", bufs=2 * NKT))
            kt_sb = [ktp.tile([128, S], BF16, tag="kt", name="kt")
                     for _ in range(B)]
            qt_sb = [qtp.tile([128, S], BF16, tag="qt", name="qt")
                     for _ in range(B)]
            vt_sb = [vtp.tile([128, S], BF16, tag="vt", name="vt")
                     for _ in range(B)]
            v_sb = [[vp.tile([128, 2 * VW], BF16, tag="v", name="v")
                     for _ in range(NKT)] for _ in range(B)]

            # ---------- pools for projections + attention -------------------
            xtp = actx.enter_context(tc.tile_pool(name="xtp", bufs=CT))
            wp = actx.enter_context(tc.tile_pool(name="wp", bufs=3 * CT))
            pps = actx.enter_context(
                tc.tile_pool(name="pps", bufs=2, space="PSUM"))
            eps_ = actx.enter_context(
                tc.tile_pool(name="spsum", bufs=2, space="PSUM"))
            ops_ = actx.enter_context(
                tc.tile_pool(name="opsum", bufs=2, space="PSUM"))
            etp = actx.enter_context(tc.tile_pool(name="etp", bufs=3))
            scrp = actx.enter_context(tc.tile_pool(name="scrp", bufs=2))
            dcp = actx.enter_context(tc.tile_pool(name="dcp", bufs=2))
            bcp = actx.enter_context(tc.tile_pool(name="bcp", bufs=4))
            tmpp = actx.enter_context(tc.tile_pool(name="tmpp", bufs=6))
            drp = actx.enter_context(
                tc.tile_pool(name="drp", bufs=8, space="DRAM"))
            if True:
                w_sb = {}
                for wname in ("wk", "wq", "wv"):
                    w_sb[wname] = []
                    for c in range(CT):
                        t = wp.tile([128, 128], BF16, tag="w", name="w")
                        nc.sync.dma_start(t[:],
                                          io[wname + "_hp"][ts(c, 128), :])
                        w_sb[wname].append(t)

                def emit_proj(b):
                    # stream this batch's x^T (the two batches share slots)
                    xt_b = []
                    for c in range(CT):
                        t = xtp.tile([128, S], BF16, tag="xt", name="xt")
                        nc.sync.dma_start(t[:], io[f"xt{b}"][ts(c, 128), :])
                        xt_b.append(t)
                    for wname, dst, bias in (("wk", kt_sb, bk_sb),
                                             ("wq", qt_sb, bq_sb),
                                             ("wv", vt_sb, bv_sb)):
                        for tt in range(TT):
                            ps = pps.tile([128, 512], F32, tag="pp",
                                          name="pp")
                            for c in range(CT):
                                nc.tensor.matmul(
                                    ps[:], w_sb[wname][c][:],
                                    xt_b[c][:, ts(tt, 512)],
                                    start=(c == 0), stop=(c == CT - 1))
                            nc.vector.tensor_scalar_add(
                                dst[b][:, ts(tt, 512)], ps[:], bias[:])

                    # V (token-major, VW-wide per head: last column = 1.0)
                    for mt in range(NKT):
                        vtile = v_sb[b][mt]
                        v3 = _r(vtile, "p (h e) -> p h e", e=VW)
                        nc.vector.memset(v3[:, :, HD:HD + 1], 1.0)
                        ps = pps.tile([128, 512], F32, tag="pp", name="pp")
                        for c in range(CT):
                            nc.tensor.matmul(
                                ps[:, 0:128], xt_b[c][:, ts(mt, 128)],
                                w_sb["wv"][c][:],
                                start=(c == 0), stop=(c == CT - 1))
                        nc.vector.scalar_tensor_tensor(
                            v3[:, :, 0:HD],
                            _r(ps[:, 0:128], "p (h e) -> p h e", e=HD),
                            1.0,
                            _r(bvbc_sb, "p (h e) -> p h e", e=HD),
                            op0=ALU.mult, op1=ALU.add)

            def emit_attn(b):
                if KPH == "ad":
                    return
                for qb in range(TT):
                    dest = b * TT + qb
                    out_ps = [ops_.tile([128, 512], F32, tag="ops",
                                        name="ops") for _ in range(2)]
                    dcol_f = dcp.tile([128, 8], F32, tag="dcf", name="dcf")
                    for kt in range(NKT):
                        s_ps = eps_.tile([128, 1024], F32, tag="sps",
                                         name="sps")
                        for half in range(2):
                            nc.tensor.matmul(
                                s_ps[:, ts(half, 512)],
                                kt_sb[b][ts(half, 64), ts(kt, 128)],
                                qt_sb[b][ts(half, 64), ts(qb, 512)],
                                start=True, stop=True)
                        et = etp.tile([128, 1024], BF16, tag="et", name="et")
                        nc.scalar.activation(et[:], s_ps[:], AF.Exp,
                                             scale=INV_SQRT_D)
                        j = kt - qb * 4
                        for half in range(2):
                            nc.tensor.matmul(
                                out_ps[half][0:VW, :],
                                v_sb[b][kt][:, half * VW:(half + 1) * VW],
                                et[:, ts(half, 512)],
                                start=(kt == 0), stop=(kt == NKT - 1))
                            if 0 <= j < 4:
                                junk = scrp.tile([128, 128], BF16,
                                                 tag="junk", name="junk")
                                nc.vector.tensor_mul(
                                    junk[:],
                                    et[:, half * 512 + j * 128:
                                       half * 512 + (j + 1) * 128],
                                    eye_sb[:])
                                nc.vector.reduce_sum(
                                    dcol_f[:, half * 4 + j:half * 4 + j + 1],
                                    junk[:], axis=mybir.AxisListType.X)
                    # correction + normalization, then ship to the exchange
                    num_sb = tmpp.tile([128, 512], F32, tag="num", name="num")
                    bc_de = bcp.tile([128, 512], F32, tag="bc", name="bc")
                    bc_ed = bcp.tile([128, 512], F32, tag="bc", name="bc")
                    for half in range(2):
                        ed_dram = drp.tile([1, 512], F32, tag="edd",
                                           name="edd")
                        nc.sync.dma_start(
                            ed_dram.rearrange("o (j p) -> o p j", p=128),
                            dcol_f[:, half * 4:half * 4 + 4])
                        evac = tmpp.tile([VW, 512], F32, tag="evac",
                                         name="evac")
                        nc.vector.tensor_copy(evac[:],
                                              out_ps[half][0:VW, :])
                        de_dram = drp.tile([1, 512], F32, tag="ded",
                                           name="ded")
                        nc.sync.dma_start(de_dram[:], evac[HD:HD + 1, :])
                        nc.sync.dma_start(num_sb[ts(half, 64), :],
                                          evac[0:HD, :])
                        nc.sync.dma_start(
                            bc_de[ts(half, 64), :],
                            de_dram[0:1, :].to_broadcast((64, 512)))
                        nc.sync.dma_start(
                            bc_ed[ts(half, 64), :],
                            ed_dram[0:1, :].to_broadcast((64, 512)))
                    rcp = tmpp.tile([128, 512], F32, tag="num", name="rcp")
                    nc.vector.reciprocal(rcp[:], bc_de[:])
                    t1 = tmpp.tile([128, 512], F32, tag="num", name="t1")
                    nc.vector.tensor_mul(t1[:],
                                         vt_sb[b][:, ts(qb, 512)], bc_ed[:])
                    t2 = tmpp.tile([128, 512], F32, tag="num", name="t2")
                    nc.vector.tensor_sub(t2[:], num_sb[:], t1[:])
                    outf = tmpp.tile([128, 512], BF16, tag="outf",
                                     name="outf")
                    nc.vector.tensor_mul(outf[:], t2[:], rcp[:])
                    nc.sync.dma_start(a2a_in[dest, :, :], outf[:])

            for b in range(B):
                emit_proj(b)
                emit_attn(b)

        if KPH == "attn":
            bp2 = ctx.enter_context(tc.tile_pool(name="bp2", bufs=2))
            bisect_out(bp2, outt_sb)
            return
        # ---------------- AllToAll exchange --------------------------------
        nc.gpsimd.collective_compute(
            "AllToAll", ALU.bypass,
            replica_groups=[list(range(N_CORES))],
            ins=[a2a_in[:]], outs=[a2a_out[:]])
        obp = ctx.enter_context(tc.tile_pool(name="obp", bufs=4))
        for i in range(CT):
            ob = obp.tile([128, QB], BF16, tag="ob", name="ob")
            nc.sync.dma_start(ob[:], a2a_out[i, :, :])
            nc.vector.tensor_copy(outt_sb[i][:], ob[:])

        if KPH == "a2a":
            bp3 = ctx.enter_context(tc.tile_pool(name="bp3", bufs=2))
            bisect_out(bp3, outt_sb)
            return
        # ---------------- FFN1 + exact GELU --------------------------------
        h1p = ctx.enter_context(tc.tile_pool(name="h1p", bufs=FT))
        h1_sb = [h1p.tile([128, QB], F32R, tag="h1", name="h1")
                 for _ in range(FT)]
        with ExitStack() as fctx:
            fps = fctx.enter_context(
                tc.tile_pool(name="fpsum", bufs=4, space="PSUM"))
            w1f = fctx.enter_context(tc.tile_pool(name="w1f", bufs=24))
            w1_cur = None
            for ft in range(FT):
                fchunk, fo = divmod(ft, 4)
                if fo == 0:
                    w1_cur = []
                    for c in range(CT):
                        t = w1f.tile([128, 512], F32R, tag="w1", name="w1")
                        nc.sync.dma_start(
                            t[:], io["w1"][ts(c, 128),
                                           fchunk * 512:(fchunk + 1) * 512])
                        w1_cur.append(t)
                ps = fps.tile([128, 512], F32, tag="fp", name="fp")
                for c in range(CT):
                    nc.tensor.matmul(
                        ps[:], w1_cur[c][:, ts(fo, 128)],
                        outt_sb[c][:],
                        start=(c == 0), stop=(c == CT - 1))
                nc.scalar.activation(h1_sb[ft][:], ps[:], AF.Gelu,
                                     bias=b1_sb[:, ft:ft + 1])

        # ---------------- FFN2 + LayerNorm ---------------------------------
        with ExitStack() as gctx:
            lcp = gctx.enter_context(tc.tile_pool(name="lcp", bufs=1))
            w2p = gctx.enter_context(tc.tile_pool(name="w2p", bufs=24))
            gps = gctx.enter_context(
                tc.tile_pool(name="gpsum", bufs=4, space="PSUM"))
            h2p = gctx.enter_context(tc.tile_pool(name="h2p", bufs=4))
            lnp = gctx.enter_context(tc.tile_pool(name="lnp", bufs=2))
            stp = gctx.enter_context(tc.tile_pool(name="stp", bufs=4))

            b2bc_sb = lcp.tile([128, D], F32)
            nc.sync.dma_start(b2bc_sb[:], io["b2_bc"][:])
            g_sb = lcp.tile([128, D], F32)
            nc.sync.dma_start(g_sb[:], io["gamma_bc"][:])
            be_sb = lcp.tile([128, D], F32)
            nc.sync.dma_start(be_sb[:], io["beta_bc"][:])

            h2_sb = [h2p.tile([128, D], F32, tag="h2s", name="h2s")
                     for _ in range(QB // 128)]
            for nh in range(2):
                w2_sb = []
                for ft in range(FT):
                    t = w2p.tile([128, 512], F32R, tag="w2", name="w2")
                    nc.sync.dma_start(t[:],
                                      io["w2"][ts(ft, 128), ts(nh, 512)])
                    w2_sb.append(t)
                for mt in range(QB // 128):
                    ps = gps.tile([128, 512], F32, tag="gp", name="gp")
                    for ft in range(FT):
                        nc.tensor.matmul(
                            ps[:], h1_sb[ft][:, ts(mt, 128)],
                            w2_sb[ft][:],
                            start=(ft == 0), stop=(ft == FT - 1))
                    nc.vector.tensor_add(h2_sb[mt][:, ts(nh, 512)], ps[:],
                                         b2bc_sb[:, ts(nh, 512)])
            for mt in range(QB // 128):
                h2 = h2_sb[mt]
                mu = stp.tile([128, 1], F32, tag="st", name="st")
                nc.vector.reduce_sum(mu[:], h2[:], axis=mybir.AxisListType.X)
                mneg = stp.tile([128, 1], F32, tag="st", name="st")
                nc.scalar.mul(mneg[:], mu[:], -1.0 / D)
                hc = lnp.tile([128, D], F32, tag="ln", name="hc")
                nc.vector.tensor_scalar_add(hc[:], h2[:], mneg[:])
                sq = lnp.tile([128, D], BF16, tag="sq", name="sq")
                ssq = stp.tile([128, 1], F32, tag="st", name="st")
                nc.scalar.activation(sq[:], hc[:], AF.Square,
                                     accum_out=ssq[:])
                std = stp.tile([128, 1], F32, tag="st", name="st")
                nc.scalar.activation(std[:], ssq[:], AF.Sqrt,
                                     scale=1.0 / D, bias=eps_sb[:])
                rstd = stp.tile([128, 1], F32, tag="st", name="st")
                nc.vector.reciprocal(rstd[:], std[:])
                yn = lnp.tile([128, D], F32, tag="ln", name="yn")
                nc.vector.scalar_tensor_tensor(
                    yn[:], hc[:], rstd[:], g_sb[:],
                    op0=ALU.mult, op1=ALU.mult)
                yf = lnp.tile([128, D], F32, tag="ln", name="yf")
                nc.vector.tensor_add(yf[:], yn[:], be_sb[:])
                nc.sync.dma_start(io["y"][ts(mt, 128), :], yf[:])


def _build():
    nc = bacc.Bacc("TRN2", target_bir_lowering=False, debug=False,
                   num_devices=N_CORES)
    io = {}

    def inp(name, shape, dt):
        io[name] = nc.dram_tensor(name, shape, dt, kind="ExternalInput").ap()

    inp("xt0", [D, S], BF16)
    inp("xt1", [D, S], BF16)
    inp("wq_hp", [D, 128], BF16)
    inp("wk_hp", [D, 128], BF16)
    inp("wv_hp", [D, 128], BF16)
    inp("w1", [D, F], F32R)
    inp("w2", [F, D], F32R)
    inp("bq_hp", [128, 1], F32)
    inp("bk_hp", [128, 1], F32)
    inp("bv_hp", [128, 1], F32)
    inp("bv_bc2", [128, 128], F32)
    inp("b1_r", [128, FT], F32)
    inp("b2_bc", [128, D], F32)
    inp("gamma_bc", [128, D], F32)
    inp("beta_bc", [128, D], F32)
    inp("eye", [128, 128], BF16)
    io["y"] = nc.dram_tensor("y", [QB, D], F32, kind="ExternalOutput").ap()

    with tile.TileContext(nc) as tc:
        _emit(tc, nc, io)
    nc.compile()
    return nc


def _get_nc():
    global _NC
    if _NC is None:
        _NC = _build()
    return _NC


def _rtf32(a):
    # round fp32 to tf32-like precision (drop 13 low mantissa bits, RN)
    b = a.view(np.uint32)
    b = (b + 0x1000) & np.uint32(0xFFFFE000)
    return b.view(np.float32)


def _prep_maps(x, Wq, bq, Wk, bk, Wv, bv, W1, b1, W2, b2, gamma, beta):
    bf = ml_dtypes.bfloat16
    f4 = np.float32

    def bc(v, n=D):
        return np.ascontiguousarray(
            np.broadcast_to(np.asarray(v, f4), (128, n)))

    xt0 = np.ascontiguousarray(np.asarray(x[0], f4).T).astype(bf)
    xt1 = np.ascontiguousarray(np.asarray(x[1], f4).T).astype(bf)
    shared = {
        "xt0": xt0, "xt1": xt1,
        "w1": _rtf32(np.ascontiguousarray(np.asarray(W1, f4))),
        "w2": _rtf32(np.ascontiguousarray(np.asarray(W2, f4))),
        "b1_r": np.ascontiguousarray(np.asarray(b1, f4).reshape(FT, 128).T),
        "b2_bc": bc(b2),
        "gamma_bc": bc(gamma),
        "beta_bc": bc(beta),
        "eye": np.eye(128, dtype=bf),
    }
    Wqf, Wkf, Wvf = (np.asarray(w, f4) for w in (Wq, Wk, Wv))
    bqf, bkf, bvf = (np.asarray(v, f4) for v in (bq, bk, bv))
    in_maps = []
    for c in range(N_CORES):
        sl = slice(c * 128, (c + 1) * 128)
        in_maps.append({
            **shared,
            "wq_hp": np.ascontiguousarray(Wqf[:, sl]).astype(bf),
            "wk_hp": np.ascontiguousarray(Wkf[:, sl]).astype(bf),
            "wv_hp": np.ascontiguousarray(Wvf[:, sl]).astype(bf),
            "bq_hp": np.ascontiguousarray(bqf[sl]).reshape(128, 1),
            "bk_hp": np.ascontiguousarray(bkf[sl]).reshape(128, 1),
            "bv_hp": np.ascontiguousarray(bvf[sl]).reshape(128, 1),
            "bv_bc2": bc(bvf[sl], 128),
        })
    return in_maps


def run_full(inputs, trace=False):
    nc = _get_nc()
    in_maps = _prep_maps(**inputs)
    res = run_bass_kernel_spmd(nc, in_maps, core_ids=list(range(N_CORES)),
                               trace=trace)
    y = np.empty((B, S, D), np.float32)
    for c in range(N_CORES):
        b, q0 = c // (N_CORES // B), (c % (N_CORES // B)) * QB
        y[b, q0:q0 + QB, :] = res.results[c]["y"]
    return y, res


def kernel(**inputs):
    y, _ = run_full(inputs, trace=False)
    return y



# revision 9
# speedup vs baseline: 1.0715x; 1.0715x over previous
"""Fused transformer block (QKV -> diag-zeroed attention -> FFN -> LayerNorm)
for Trainium2, head-sharded over 8 NeuronCores with an AllToAll.

Sharding: core c owns head pair c (heads 2c, 2c+1) for attention over ALL
tokens of both batches. The attention outputs are exchanged with a single
AllToAll so core c then owns token block c (batch c//4, queries
[(c%4)*512, ...+512)) with the full model dim, and runs FFN + LayerNorm.

Attention math (diagonal zeroed *after* softmax, per reference):
  eT[k, q] = exp(score[k, q] / 32)             (scores are small: no max-sub)
  num[d, q] = sum_k eT[k,q] V[k,d]  -  eT[q,q] V[q,d]   (both inside PSUM:
      the diagonal term is subtracted by an extra matmul whose moving side
      is  et ⊙ (−I)  for the diagonal 128-tile)
  denom[q]  = ones-column of the V matmul (keeps the diagonal, as softmax)
  out[d, q] = num[d, q] * (1/denom[q])          (reciprocal row broadcast
      across partitions by the Pool engine's partition_broadcast)

Schedule notes:
 - scores(kt+1) is emitted before AV(kt) so the PE keeps running while the
   Act engine computes exp(kt); the attention phase is Act(exp)-bound.
 - W1/W2 are bf16 and prefetched (W1 during attention on the DVE queue,
   W2 during the AllToAll window) so the FFN never waits on HBM.
 - FFN2 is token-block-outer with LayerNorm pipelined per 128-token block.
"""

import numpy as np
import ml_dtypes
from contextlib import ExitStack

import concourse.bass as bass
import concourse.mybir as mybir
import concourse.tile as tile
from concourse import bacc
from concourse.bass_utils import run_bass_kernel_spmd

AF = mybir.ActivationFunctionType
ALU = mybir.AluOpType
BF16 = mybir.dt.bfloat16
F32 = mybir.dt.float32

N_CORES = 8
B, S, D, H, HD, F = 2, 2048, 1024, 16, 64, 4096
QB = 512          # tokens per core after the exchange
CT = D // 128     # 8 contraction tiles over D
TT = S // 512     # 4 token 512-blocks per batch
NKT = S // 128    # 16 key 128-tiles per batch
FT = F // 128     # 32 f 128-tiles
INV_SQRT_D = 1.0 / 32.0
LN_EPS = 1e-5
VW = HD + 1       # 65: V columns per head incl. the ones column

_NC = None
ts = bass.ts


def _emit(tc, nc, io):
    with ExitStack() as ctx:
        # ---------------- constants ----------------------------------------
        cpool = ctx.enter_context(tc.tile_pool(name="consts", bufs=1))
        neye_sb = cpool.tile([128, 128], BF16)
        nc.sync.dma_start(neye_sb[:], io["neg_eye"][:])
        bq_sb = cpool.tile([128, 1], F32)
        nc.sync.dma_start(bq_sb[:], io["bq_hp"][:])
        bk_sb = cpool.tile([128, 1], F32)
        nc.sync.dma_start(bk_sb[:], io["bk_hp"][:])
        bvbc_sb = cpool.tile([128, 128], F32)
        nc.sync.dma_start(bvbc_sb[:], io["bv_bc2"][:])
        b1_sb = cpool.tile([128, FT], F32)
        nc.sync.dma_start(b1_sb[:], io["b1_r"][:])
        eps_sb = cpool.tile([128, 1], F32)
        nc.vector.memset(eps_sb[:], LN_EPS)

        # W1 resident in SBUF through FFN1 (prefetched during the
        # projection/attention phase; released before the LN pools open).
        w1ctx = ExitStack()
        w1p = w1ctx.enter_context(tc.tile_pool(name="w1p", bufs=CT,
                                               side="right"))
        w1_sb = [w1p.tile([128, F], BF16, tag="w1", name="w1")
                 for _ in range(CT)]

        a2a_in = nc.dram_tensor("a2a_in", [N_CORES, 128, QB], BF16).ap()
        a2a_out = nc.dram_tensor("a2a_out", [N_CORES, 128, QB], BF16).ap()

        # ---------------- projections + attention --------------------------
        with ExitStack() as actx:
            ktp = actx.enter_context(tc.tile_pool(name="ktp", bufs=2))
            qtp = actx.enter_context(tc.tile_pool(name="qtp", bufs=2))
            vp = actx.enter_context(tc.tile_pool(name="vp", bufs=2 * NKT))
            kt_sb = [ktp.tile([128, S], BF16, tag="kt", name="kt")
                     for _ in range(B)]
            qt_sb = [qtp.tile([128, S], BF16, tag="qt", name="qt")
                     for _ in range(B)]
            v_sb = [[vp.tile([128, 2 * VW], BF16, tag="v", name="v")
                     for _ in range(NKT)] for _ in range(B)]

            wp = actx.enter_context(tc.tile_pool(name="wp", bufs=3 * CT))
            w_sb = {}
            for wname in ("wk", "wq", "wv"):
                w_sb[wname] = []
                for c in range(CT):
                    t = wp.tile([128, 128], BF16, tag="w", name="w")
                    nc.sync.dma_start(t[:], io[wname + "_hp"][ts(c, 128), :])
                    w_sb[wname].append(t)

            # -------- all projections up front (own PSUM scope) ------------
            with ExitStack() as pctx:
                xtp = pctx.enter_context(tc.tile_pool(name="xtp", bufs=16))
                pps = pctx.enter_context(
                    tc.tile_pool(name="pps", bufs=3, space="PSUM"))
                vps = pctx.enter_context(
                    tc.tile_pool(name="vps", bufs=2, space="PSUM"))
                for b in range(B):
                    for tt in range(TT):
                        xt_t = []
                        for c in range(CT):
                            t = xtp.tile([128, 512], BF16, tag="xt",
                                         name="xt")
                            nc.sync.dma_start(
                                t[:], io[f"xt{b}"][ts(c, 128), ts(tt, 512)])
                            xt_t.append(t)
                        for wname, dst, bias in (("wk", kt_sb, bk_sb),
                                                 ("wq", qt_sb, bq_sb)):
                            ps = pps.tile([128, 512], F32, tag="pp",
                                          name="pp")
                            for c in range(CT):
                                nc.tensor.matmul(
                                    ps[:], w_sb[wname][c][:], xt_t[c][:],
                                    start=(c == 0), stop=(c == CT - 1))
                            nc.vector.tensor_scalar_add(
                                dst[b][:, ts(tt, 512)], ps[:], bias[:])
                        # token-major V for the 4 key-tiles of this block
                        for mo in range(4):
                            mt = tt * 4 + mo
                            vtile = v_sb[b][mt]
                            v3 = vtile.rearrange("p (h e) -> p h e", e=VW)
                            nc.vector.memset(v3[:, :, HD:HD + 1], 1.0)
                            ps = vps.tile([128, 128], F32, tag="vp",
                                          name="vp")
                            for c in range(CT):
                                nc.tensor.matmul(
                                    ps[:], xt_t[c][:, ts(mo, 128)],
                                    w_sb["wv"][c][:],
                                    start=(c == 0), stop=(c == CT - 1))
                            nc.vector.scalar_tensor_tensor(
                                v3[:, :, 0:HD],
                                ps.rearrange("p (h e) -> p h e", e=HD),
                                1.0,
                                bvbc_sb.rearrange("p (h e) -> p h e", e=HD),
                                op0=ALU.mult, op1=ALU.add)

            # W1 prefetch on the SP queue right after the xt loads; lands
            # well before FFN1 needs it.
            for c in range(CT):
                nc.sync.dma_start(w1_sb[c][:], io["w1"][ts(c, 128), :])

            # -------- attention (PSUM: 2x scores + 4x out = 8 banks) -------
            sps = actx.enter_context(
                tc.tile_pool(name="sps", bufs=2, space="PSUM"))
            ops = actx.enter_context(
                tc.tile_pool(name="ops", bufs=4, space="PSUM"))
            etp = actx.enter_context(tc.tile_pool(name="etp", bufs=3))
            jkp = actx.enter_context(tc.tile_pool(name="jkp", bufs=3))
            rcp = actx.enter_context(tc.tile_pool(name="rcp", bufs=2))
            bcp = actx.enter_context(tc.tile_pool(name="bcp", bufs=3))
            stp = actx.enter_context(tc.tile_pool(name="stp", bufs=3))

            for b in range(B):
                for qb in range(TT):
                    dest = b * TT + qb

                    def scores(kt):
                        sp = sps.tile([128, 1024], F32, tag="sp", name="sp")
                        for half in range(2):
                            nc.tensor.matmul(
                                sp[:, ts(half, 512)],
                                kt_sb[b][ts(half, 64), ts(kt, 128)],
                                qt_sb[b][ts(half, 64), ts(qb, 512)],
                                start=True, stop=True)
                        return sp

                    out_h = [ops.tile([VW, 512], F32, tag="op", name="op",
                                      padded_shape=[128, 512])
                             for _ in range(2)]
                    sp_cur = scores(0)
                    for kt in range(NKT):
                        et = etp.tile([128, 1024], BF16, tag="et", name="et")
                        nc.scalar.activation(et[:], sp_cur[:], AF.Exp,
                                             scale=INV_SQRT_D)
                        if kt + 1 < NKT:
                            sp_cur = scores(kt + 1)
                        j = kt - qb * 4
                        diag = 0 <= j < 4
                        junk = []
                        if diag:
                            for h in range(2):
                                jk = jkp.tile([128, 128], BF16, tag="jk",
                                              name="jk")
                                nc.vector.tensor_mul(
                                    jk[:],
                                    et[:, h * 512 + j * 128:
                                       h * 512 + (j + 1) * 128],
                                    neye_sb[:])
                                junk.append(jk)
                        last_av = (kt == NKT - 1)
                        for h in range(2):
                            nc.tensor.matmul(
                                out_h[h][0:VW, :],
                                v_sb[b][kt][:, h * VW:(h + 1) * VW],
                                et[:, ts(h, 512)],
                                start=(kt == 0),
                                stop=(last_av and not diag))
                        if diag:
                            for h in range(2):
                                nc.tensor.matmul(
                                    out_h[h][0:HD, ts(j, 128)],
                                    v_sb[b][kt][:, h * VW:h * VW + HD],
                                    junk[h][:],
                                    start=False, stop=last_av)

                    stage = stp.tile([128, QB], BF16, tag="st", name="st")
                    for h in range(2):
                        rr = rcp.tile([1, QB], F32, tag="rr", name="rr")
                        nc.vector.reciprocal(rr[:], out_h[h][HD:HD + 1, :])
                        bc = bcp.tile([HD, QB], F32, tag="bc", name="bc")
                        nc.gpsimd.partition_broadcast(bc[:], rr[:])
                        nc.vector.tensor_mul(stage[ts(h, HD), :],
                                             out_h[h][0:HD, :], bc[:])
                    nc.sync.dma_start(a2a_in[dest, :, :], stage[:])

        # ---------------- AllToAll exchange --------------------------------
        nc.gpsimd.collective_compute(
            "AllToAll", ALU.bypass,
            replica_groups=[list(range(N_CORES))],
            ins=[a2a_in[:]], outs=[a2a_out[:]])

        # ---------------- FFN1 + exact GELU --------------------------------
        with ExitStack() as fctx:
            w2p = fctx.enter_context(tc.tile_pool(name="w2p", bufs=FT))
            w2_sb = [w2p.tile([128, D], BF16, tag="w2", name="w2")
                     for _ in range(FT)]
            # W2 on the Act queue: dispatches right after the last exp, so
            # the 8 MB stream overlaps the AllToAll window.
            for ft in range(FT):
                nc.scalar.dma_start(w2_sb[ft][:], io["w2"][ts(ft, 128), :])

            lcp = fctx.enter_context(tc.tile_pool(name="lcp", bufs=1))
            b2bc_sb = lcp.tile([128, D], F32)
            nc.sync.dma_start(b2bc_sb[:], io["b2_bc"][:])
            g_sb = lcp.tile([128, D], F32)
            nc.sync.dma_start(g_sb[:], io["gamma_bc"][:])
            be_sb = lcp.tile([128, D], F32)
            nc.sync.dma_start(be_sb[:], io["beta_bc"][:])

            h1p = fctx.enter_context(tc.tile_pool(name="h1p", bufs=FT))
            h1_sb = [h1p.tile([128, QB], BF16, tag="h1", name="h1")
                     for _ in range(FT)]
            with ExitStack() as f1ctx:
                obp = f1ctx.enter_context(tc.tile_pool(name="obp", bufs=1))
                outt = obp.tile([128, N_CORES * QB], BF16)
                nc.sync.dma_start(
                    outt.rearrange("p (c q) -> p c q", c=N_CORES),
                    a2a_out[:].rearrange("c p q -> p c q"))
                fps = f1ctx.enter_context(
                    tc.tile_pool(name="fps", bufs=4, space="PSUM"))
                outt_v = outt.rearrange("p (c q) -> p c q", c=N_CORES)
                for ft in range(FT):
                    ps = fps.tile([128, 512], F32, tag="fp", name="fp")
                    for c in range(CT):
                        nc.tensor.matmul(
                            ps[:], w1_sb[c][:, ts(ft, 128)], outt_v[:, c, :],
                            start=(c == 0), stop=(c == CT - 1))
                    nc.scalar.activation(h1_sb[ft][:], ps[:], AF.Gelu,
                                         bias=b1_sb[:, ft:ft + 1])
            w1ctx.close()

            # ------------- FFN2 + LayerNorm, per 128-token block -----------
            gps = fctx.enter_context(
                tc.tile_pool(name="gps", bufs=4, space="PSUM"))
            h2p = fctx.enter_context(tc.tile_pool(name="h2p", bufs=2))
            lnp = fctx.enter_context(tc.tile_pool(name="lnp", bufs=4))
            sstp = fctx.enter_context(tc.tile_pool(name="sstp", bufs=8))
            for mt in range(QB // 128):
                h2 = h2p.tile([128, D], F32, tag="h2", name="h2")
                for nh in range(2):
                    ps = gps.tile([128, 512], F32, tag="gp", name="gp")
                    for ft in range(FT):
                        nc.tensor.matmul(
                            ps[:], h1_sb[ft][:, ts(mt, 128)],
                            w2_sb[ft][:, ts(nh, 512)],
                            start=(ft == 0), stop=(ft == FT - 1))
                    nc.vector.tensor_add(h2[:, ts(nh, 512)], ps[:],
                                         b2bc_sb[:, ts(nh, 512)])
                mu = sstp.tile([128, 1], F32, tag="ss", name="ss")
                nc.vector.reduce_sum(mu[:], h2[:], axis=mybir.AxisListType.X)
                mneg = sstp.tile([128, 1], F32, tag="ss", name="ss")
                nc.scalar.mul(mneg[:], mu[:], -1.0 / D)
                hc = lnp.tile([128, D], F32, tag="ln", name="hc")
                nc.vector.tensor_scalar_add(hc[:], h2[:], mneg[:])
                sq = lnp.tile([128, D], BF16, tag="sq", name="sq")
                ssq = sstp.tile([128, 1], F32, tag="ss", name="ss")
                nc.scalar.activation(sq[:], hc[:], AF.Square,
                                     accum_out=ssq[:])
                std = sstp.tile([128, 1], F32, tag="ss", name="ss")
                nc.scalar.activation(std[:], ssq[:], AF.Sqrt,
                                     scale=1.0 / D, bias=eps_sb[:])
                rstd = sstp.tile([128, 1], F32, tag="ss", name="ss")
                nc.vector.reciprocal(rstd[:], std[:])
                yn = lnp.tile([128, D], F32, tag="ln", name="yn")
                nc.vector.scalar_tensor_tensor(
                    yn[:], hc[:], rstd[:], g_sb[:],
                    op0=ALU.mult, op1=ALU.mult)
                yf = lnp.tile([128, D], F32, tag="ln", name="yf")
                nc.vector.tensor_add(yf[:], yn[:], be_sb[:])
                nc.sync.dma_start(io["y"][ts(mt, 128), :], yf[:])


def _build():
    nc = bacc.Bacc("TRN2", target_bir_lowering=False, debug=False,
                   num_devices=N_CORES)
    io = {}

    def inp(name, shape, dt):
        io[name] = nc.dram_tensor(name, shape, dt, kind="ExternalInput").ap()

    inp("xt0", [D, S], BF16)
    inp("xt1", [D, S], BF16)
    inp("wq_hp", [D, 128], BF16)
    inp("wk_hp", [D, 128], BF16)
    inp("wv_hp", [D, 128], BF16)
    inp("w1", [D, F], BF16)
    inp("w2", [F, D], BF16)
    inp("bq_hp", [128, 1], F32)
    inp("bk_hp", [128, 1], F32)
    inp("bv_bc2", [128, 128], F32)
    inp("b1_r", [128, FT], F32)
    inp("b2_bc", [128, D], F32)
    inp("gamma_bc", [128, D], F32)
    inp("beta_bc", [128, D], F32)
    inp("neg_eye", [128, 128], BF16)
    io["y"] = nc.dram_tensor("y", [QB, D], F32, kind="ExternalOutput").ap()

    with tile.TileContext(nc) as tc:
        _emit(tc, nc, io)
    nc.compile()
    return nc


def _get_nc():
    global _NC
    if _NC is None:
        _NC = _build()
    return _NC


def _prep_maps(x, Wq, bq, Wk, bk, Wv, bv, W1, b1, W2, b2, gamma, beta):
    bf = ml_dtypes.bfloat16
    f4 = np.float32

    def bc(v, n=D):
        return np.ascontiguousarray(
            np.broadcast_to(np.asarray(v, f4), (128, n)))

    xt0 = np.ascontiguousarray(np.asarray(x[0], f4).T).astype(bf)
    xt1 = np.ascontiguousarray(np.asarray(x[1], f4).T).astype(bf)
    shared = {
        "xt0": xt0, "xt1": xt1,
        "w1": np.ascontiguousarray(np.asarray(W1, f4)).astype(bf),
        "w2": np.ascontiguousarray(np.asarray(W2, f4)).astype(bf),
        "b1_r": np.ascontiguousarray(np.asarray(b1, f4).reshape(FT, 128).T),
        "b2_bc": bc(b2),
        "gamma_bc": bc(gamma),
        "beta_bc": bc(beta),
        "neg_eye": (-np.eye(128)).astype(bf),
    }
    Wqf, Wkf, Wvf = (np.asarray(w, f4) for w in (Wq, Wk, Wv))
    bqf, bkf, bvf = (np.asarray(v, f4) for v in (bq, bk, bv))
    in_maps = []
    for c in range(N_CORES):
        sl = slice(c * 128, (c + 1) * 128)
        in_maps.append({
            **shared,
            "wq_hp": np.ascontiguousarray(Wqf[:, sl]).astype(bf),
            "wk_hp": np.ascontiguousarray(Wkf[:, sl]).astype(bf),
            "wv_hp": np.ascontiguousarray(Wvf[:, sl]).astype(bf),
            "bq_hp": np.ascontiguousarray(bqf[sl]).reshape(128, 1),
            "bk_hp": np.ascontiguousarray(bkf[sl]).reshape(128, 1),
            "bv_bc2": bc(bvf[sl], 128),
        })
    return in_maps


def run_full(inputs, trace=False):
    nc = _get_nc()
    in_maps = _prep_maps(**inputs)
    res = run_bass_kernel_spmd(nc, in_maps, core_ids=list(range(N_CORES)),
                               trace=trace)
    y = np.empty((B, S, D), np.float32)
    for c in range(N_CORES):
        b, q0 = c // (N_CORES // B), (c % (N_CORES // B)) * QB
        y[b, q0:q0 + QB, :] = res.results[c]["y"]
    return y, res


def kernel(**inputs):
    y, _ = run_full(inputs, trace=False)
    return y


# revision 15
# speedup vs baseline: 1.1431x; 1.0668x over previous
"""Fused transformer block (QKV -> diag-zeroed attention -> FFN -> LayerNorm)
for Trainium2, head-sharded over 8 NeuronCores with an AllToAll.

Sharding: core c owns head pair c (heads 2c, 2c+1) for attention over ALL
tokens of both batches. The attention outputs are exchanged with a single
AllToAll so core c then owns token block c (batch c//4, queries
[(c%4)*512, ...+512)) with the full model dim, and runs FFN + LayerNorm.

Attention math (diagonal zeroed *after* softmax, per reference):
  eT[k, q] = exp(score[k, q] / 32)             (scores are small: no max-sub)
  num[d, q] = sum_k eT[k,q] V[k,d]  -  eT[q,q] V[q,d]   (the diagonal term
      is subtracted inside PSUM by an extra matmul whose moving side is
      et ⊙ (−I) for the diagonal 128-tile)
  denom[q]  = ones-column of the V matmul (keeps the diagonal, as softmax)
  out[d, q] = num[d, q] * (1/denom[q])          (reciprocal row broadcast
      across partitions by the Pool engine's partition_broadcast)

Schedule:
 - batch-0 K/Q/V projections first (V computed d-major with 512-wide
   matmuls, then PE-transposed to token-major), then the 8 attention
   blocks; batch-1's projections are interleaved into batch-0's blocks so
   they hide under the Act-bound exp stream.
 - scores(next block, kt=0) is hoisted before AV(kt=15) so neither PE nor
   Act stalls at block boundaries.
 - W1/W2 are bf16: W1 prefetched during attention, W2 during the AllToAll
   window; FFN2+LayerNorm pipelined per 128-token block.
"""

import numpy as np
import ml_dtypes
from contextlib import ExitStack

import concourse.bass as bass
import concourse.mybir as mybir
import concourse.tile as tile
from concourse import bacc
from concourse.bass_utils import run_bass_kernel_spmd

AF = mybir.ActivationFunctionType
ALU = mybir.AluOpType
BF16 = mybir.dt.bfloat16
F32 = mybir.dt.float32

N_CORES = 8
B, S, D, H, HD, F = 2, 2048, 1024, 16, 64, 4096
QB = 512          # tokens per core after the exchange
CT = D // 128     # 8 contraction tiles over D
TT = S // 512     # 4 token 512-blocks per batch
NKT = S // 128    # 16 key 128-tiles per batch
FT = F // 128     # 32 f 128-tiles
INV_SQRT_D = 1.0 / 32.0
LN_EPS = 1e-5
VW = HD + 1       # 65: V columns per head incl. the ones column

_NC = None
ts = bass.ts


def _emit(tc, nc, io):
    with ExitStack() as ctx:
        # -------- packed constants (2 DMAs on the Act queue) ----------------
        cpool = ctx.enter_context(tc.tile_pool(name="consts", bufs=1))
        wpack = cpool.tile([128, 26 * 128], BF16)
        nc.scalar.dma_start(wpack[:], io["wpack"][:])
        w_sb = {n: [wpack[:, ts(i * CT + c, 128)] for c in range(CT)]
                for i, n in enumerate(("wk", "wq", "wv"))}
        eye_sb = wpack[:, ts(24, 128)]
        neye_sb = wpack[:, ts(25, 128)]
        fpack = cpool.tile([128, 3 + FT], F32)
        nc.scalar.dma_start(fpack[:], io["fpack"][:])
        bk_sb = fpack[:, 0:1]
        bq_sb = fpack[:, 1:2]
        bv_sb = fpack[:, 2:3]
        b1_sb = fpack[:, 3:3 + FT]
        eps_sb = cpool.tile([128, 1], F32)
        nc.vector.memset(eps_sb[:], LN_EPS)

        # W1 resident through FFN1 (right-side stack so it can release
        # before the LN pools open). Loads go last on the SP queue.
        w1ctx = ExitStack()
        w1p = w1ctx.enter_context(tc.tile_pool(name="w1p", bufs=CT,
                                               side="right"))
        w1_sb = [w1p.tile([128, F], BF16, tag="w1", name="w1")
                 for _ in range(CT)]

        a2a_in = nc.dram_tensor("a2a_in", [N_CORES, 128, QB], BF16).ap()
        a2a_out = nc.dram_tensor("a2a_out", [N_CORES, 128, QB], BF16).ap()

        with ExitStack() as actx:
            ktp = actx.enter_context(tc.tile_pool(name="ktp", bufs=2))
            qtp = actx.enter_context(tc.tile_pool(name="qtp", bufs=2))
            vp = actx.enter_context(tc.tile_pool(name="vp", bufs=2 * NKT))
            xtp = actx.enter_context(tc.tile_pool(name="xtp", bufs=2 * CT))
            vdp = actx.enter_context(tc.tile_pool(name="vdp", bufs=2))
            kt_sb = [ktp.tile([128, S], BF16, tag="kt", name="kt")
                     for _ in range(B)]
            qt_sb = [qtp.tile([128, S], BF16, tag="qt", name="qt")
                     for _ in range(B)]
            v_sb = [[vp.tile([128, 2 * VW], BF16, tag="v", name="v")
                     for _ in range(NKT)] for _ in range(B)]
            xt_sb = [[None] * CT for _ in range(B)]

            def load_xt(b):
                for c in range(CT):
                    t = xtp.tile([128, S], BF16, tag="xt", name="xt")
                    nc.sync.dma_start(t[:], io[f"xt{b}"][ts(c, 128), :])
                    xt_sb[b][c] = t

            def kq_chain(b, tt, pool, wname, dst, bias):
                ps = pool.tile([128, 512], F32, tag="pp", name="pp")
                for c in range(CT):
                    nc.tensor.matmul(
                        ps[:], w_sb[wname][c], xt_sb[b][c][:, ts(tt, 512)],
                        start=(c == 0), stop=(c == CT - 1))
                nc.vector.tensor_scalar_add(
                    dst[b][:, ts(tt, 512)], ps[:], bias)

            def v_group(b, tt, pool, on_act):
                # d-major V chain, then 4 PE transposes to token-major
                ps = pool.tile([128, 512], F32, tag="pp", name="pp")
                for c in range(CT):
                    nc.tensor.matmul(
                        ps[:], w_sb["wv"][c], xt_sb[b][c][:, ts(tt, 512)],
                        start=(c == 0), stop=(c == CT - 1))
                vd = vdp.tile([128, 512], BF16, tag="vd", name="vd")
                nc.vector.tensor_scalar_add(vd[:], ps[:], bv_sb)
                for mo in range(4):
                    mt = tt * 4 + mo
                    pst = pool.tile([128, 128], BF16, tag="pt", name="pt",
                                    bufs=2)
                    nc.tensor.transpose(pst[:], vd[:, ts(mo, 128)], eye_sb)
                    vtile = v_sb[b][mt]
                    v3 = vtile.rearrange("p (h e) -> p h e", e=VW)
                    nc.vector.memset(v3[:, :, HD:HD + 1], 1.0)
                    src = pst.rearrange("p (h e) -> p h e", e=HD)
                    if on_act:
                        nc.scalar.activation(v3[:, :, 0:HD], src, AF.Copy)
                    else:
                        nc.vector.tensor_copy(v3[:, :, 0:HD], src)

            # -------- batch-0 proj + both batches' V (own PSUM scope) -------
            load_xt(0)
            with ExitStack() as pctx:
                ppsA = pctx.enter_context(
                    tc.tile_pool(name="ppsA", bufs=5, space="PSUM"))
                for tt in range(TT):
                    kq_chain(0, tt, ppsA, "wk", kt_sb, bk_sb)
                    kq_chain(0, tt, ppsA, "wq", qt_sb, bq_sb)
                load_xt(1)
                for tt in range(TT):
                    v_group(0, tt, ppsA, on_act=True)
                for tt in range(TT):
                    v_group(1, tt, ppsA, on_act=True)

            # W1 prefetch streams behind the xt loads on SP
            for c in range(CT):
                nc.sync.dma_start(w1_sb[c][:], io["w1"][ts(c, 128), :])

            # -------- attention: 8 blocks, batch-1 proj interleaved ---------
            sps = actx.enter_context(
                tc.tile_pool(name="sps", bufs=2, space="PSUM"))
            ops = actx.enter_context(
                tc.tile_pool(name="ops", bufs=3, space="PSUM"))
            ppsB = actx.enter_context(
                tc.tile_pool(name="ppsB", bufs=1, space="PSUM"))
            etp = actx.enter_context(tc.tile_pool(name="etp", bufs=3))
            jkp = actx.enter_context(tc.tile_pool(name="jkp", bufs=3))
            rcp = actx.enter_context(tc.tile_pool(name="rcp", bufs=2))
            bcp = actx.enter_context(tc.tile_pool(name="bcp", bufs=3))
            stp = actx.enter_context(tc.tile_pool(name="stp", bufs=3))

            proj_tasks = []
            for tt in range(TT):
                proj_tasks.append(
                    lambda tt=tt: kq_chain(1, tt, ppsB, "wk", kt_sb, bk_sb))
                proj_tasks.append(
                    lambda tt=tt: kq_chain(1, tt, ppsB, "wq", qt_sb, bq_sb))

            blocks = [(b, qb) for b in range(B) for qb in range(TT)]

            def scores(b, qb, kt):
                sp = sps.tile([128, 1024], F32, tag="sp", name="sp")
                for half in range(2):
                    nc.tensor.matmul(
                        sp[:, ts(half, 512)],
                        kt_sb[b][ts(half, 64), ts(kt, 128)],
                        qt_sb[b][ts(half, 64), ts(qb, 512)],
                        start=True, stop=True)
                return sp

            sp_cur = scores(0, 0, 0)
            for bi, (b, qb) in enumerate(blocks):
                dest = b * TT + qb
                out_h = [ops.tile([VW, 512], F32, tag="op", name="op")
                         for _ in range(2)]
                sp_next = None
                for kt in range(NKT):
                    et = etp.tile([128, 1024], BF16, tag="et", name="et")
                    nc.scalar.activation(et[:], sp_cur[:], AF.Exp,
                                         scale=INV_SQRT_D)
                    if kt + 1 < NKT:
                        sp_cur = scores(b, qb, kt + 1)
                    elif bi + 1 < len(blocks):
                        nb, nqb = blocks[bi + 1]
                        sp_next = scores(nb, nqb, 0)
                    j = kt - qb * 4
                    diag = 0 <= j < 4
                    junk = []
                    if diag:
                        for h in range(2):
                            jk = jkp.tile([128, 128], BF16, tag="jk",
                                          name="jk")
                            nc.vector.tensor_mul(
                                jk[:],
                                et[:, h * 512 + j * 128:
                                   h * 512 + (j + 1) * 128],
                                neye_sb)
                            junk.append(jk)
                    last_av = (kt == NKT - 1)
                    for h in range(2):
                        nc.tensor.matmul(
                            out_h[h][0:VW, :],
                            v_sb[b][kt][:, h * VW:(h + 1) * VW],
                            et[:, ts(h, 512)],
                            start=(kt == 0),
                            stop=(last_av and not diag))
                    if diag:
                        for h in range(2):
                            nc.tensor.matmul(
                                out_h[h][0:HD, ts(j, 128)],
                                v_sb[b][kt][:, h * VW:h * VW + HD],
                                junk[h][:],
                                start=False, stop=last_av)
                sp_cur = sp_next

                # batch-1 K/Q projections hide under batch-0's exp stream
                if b == 0 and qb < 3:
                    for _ in range(3 if qb < 2 else 2):
                        proj_tasks.pop(0)()

                stage = stp.tile([128, QB], BF16, tag="st", name="st")
                for h in range(2):
                    rr = rcp.tile([1, QB], F32, tag="rr", name="rr")
                    nc.vector.reciprocal(rr[:], out_h[h][HD:HD + 1, :])
                    bc = bcp.tile([HD, QB], F32, tag="bc", name="bc")
                    nc.gpsimd.partition_broadcast(bc[:], rr[:])
                    nc.vector.tensor_mul(stage[ts(h, HD), :],
                                         out_h[h][0:HD, :], bc[:])
                nc.gpsimd.dma_start(a2a_in[dest, :, :], stage[:])

        # ---------------- AllToAll exchange --------------------------------
        nc.gpsimd.collective_compute(
            "AllToAll", ALU.bypass,
            replica_groups=[list(range(N_CORES))],
            ins=[a2a_in[:]], outs=[a2a_out[:]])

        # ---------------- FFN1 + exact GELU --------------------------------
        with ExitStack() as fctx:
            w2p = fctx.enter_context(tc.tile_pool(name="w2p", bufs=FT))
            w2_sb = [w2p.tile([128, D], BF16, tag="w2", name="w2")
                     for _ in range(FT)]
            # W2 on the Act queue: dispatches right after the last exp, so
            # the 8 MB stream overlaps the AllToAll window.
            for ft in range(FT):
                nc.scalar.dma_start(w2_sb[ft][:], io["w2"][ts(ft, 128), :])

            lcp = fctx.enter_context(tc.tile_pool(name="lcp", bufs=1))
            lnpack = lcp.tile([128, 3 * D], F32)
            nc.sync.dma_start(lnpack[:], io["lnpack"][:])
            b2bc_sb = lnpack[:, 0:D]
            g_sb = lnpack[:, D:2 * D]
            be_sb = lnpack[:, 2 * D:3 * D]

            h1p = fctx.enter_context(tc.tile_pool(name="h1p", bufs=FT))
            h1_sb = [h1p.tile([128, QB], BF16, tag="h1", name="h1")
                     for _ in range(FT)]
            with ExitStack() as f1ctx:
                obp = f1ctx.enter_context(tc.tile_pool(name="obp", bufs=1))
                outt = obp.tile([128, N_CORES * QB], BF16)
                nc.sync.dma_start(
                    outt.rearrange("p (c q) -> p c q", c=N_CORES),
                    a2a_out[:].rearrange("c p q -> p c q"))
                fps = f1ctx.enter_context(
                    tc.tile_pool(name="fps", bufs=4, space="PSUM"))
                outt_v = outt.rearrange("p (c q) -> p c q", c=N_CORES)
                for ft in range(FT):
                    ps = fps.tile([128, 512], F32, tag="fp", name="fp")
                    for c in range(CT):
                        nc.tensor.matmul(
                            ps[:], w1_sb[c][:, ts(ft, 128)], outt_v[:, c, :],
                            start=(c == 0), stop=(c == CT - 1))
                    nc.scalar.activation(h1_sb[ft][:], ps[:], AF.Gelu,
                                         bias=b1_sb[:, ft:ft + 1])
            w1ctx.close()

            # ------------- FFN2 + LayerNorm, per 128-token block -----------
            gps = fctx.enter_context(
                tc.tile_pool(name="gps", bufs=4, space="PSUM"))
            h2p = fctx.enter_context(tc.tile_pool(name="h2p", bufs=2))
            lnp = fctx.enter_context(tc.tile_pool(name="lnp", bufs=4))
            sstp = fctx.enter_context(tc.tile_pool(name="sstp", bufs=8))
            for mt in range(QB // 128):
                h2 = h2p.tile([128, D], F32, tag="h2", name="h2")
                for nh in range(2):
                    ps = gps.tile([128, 512], F32, tag="gp", name="gp")
                    for ft in range(FT):
                        nc.tensor.matmul(
                            ps[:], h1_sb[ft][:, ts(mt, 128)],
                            w2_sb[ft][:, ts(nh, 512)],
                            start=(ft == 0), stop=(ft == FT - 1))
                    nc.vector.tensor_add(h2[:, ts(nh, 512)], ps[:],
                                         b2bc_sb[:, ts(nh, 512)])
                mu = sstp.tile([128, 1], F32, tag="ss", name="ss")
                nc.vector.reduce_sum(mu[:], h2[:], axis=mybir.AxisListType.X)
                mneg = sstp.tile([128, 1], F32, tag="ss", name="ss")
                nc.scalar.mul(mneg[:], mu[:], -1.0 / D)
                hc = lnp.tile([128, D], F32, tag="ln", name="hc")
                nc.vector.tensor_scalar_add(hc[:], h2[:], mneg[:])
                sq = lnp.tile([128, D], BF16, tag="sq", name="sq")
                ssq = sstp.tile([128, 1], F32, tag="ss", name="ss")
                nc.scalar.activation(sq[:], hc[:], AF.Square,
                                     accum_out=ssq[:])
                std = sstp.tile([128, 1], F32, tag="ss", name="ss")
                nc.scalar.activation(std[:], ssq[:], AF.Sqrt,
                                     scale=1.0 / D, bias=eps_sb[:])
                rstd = sstp.tile([128, 1], F32, tag="ss", name="ss")
                nc.vector.reciprocal(rstd[:], std[:])
                yn = lnp.tile([128, D], F32, tag="ln", name="yn")
                nc.vector.scalar_tensor_tensor(
                    yn[:], hc[:], rstd[:], g_sb[:],
                    op0=ALU.mult, op1=ALU.mult)
                yf = lnp.tile([128, D], F32, tag="ln", name="yf")
                nc.vector.tensor_add(yf[:], yn[:], be_sb[:])
                nc.sync.dma_start(io["y"][ts(mt, 128), :], yf[:])


def _build():
    nc = bacc.Bacc("TRN2", target_bir_lowering=False, debug=False,
                   num_devices=N_CORES)
    io = {}

    def inp(name, shape, dt):
        io[name] = nc.dram_tensor(name, shape, dt, kind="ExternalInput").ap()

    inp("xt0", [D, S], BF16)
    inp("xt1", [D, S], BF16)
    inp("wpack", [128, 26 * 128], BF16)
    inp("fpack", [128, 3 + FT], F32)
    inp("w1", [D, F], BF16)
    inp("w2", [F, D], BF16)
    inp("lnpack", [128, 3 * D], F32)
    io["y"] = nc.dram_tensor("y", [QB, D], F32, kind="ExternalOutput").ap()

    with tile.TileContext(nc) as tc:
        _emit(tc, nc, io)
    nc.compile()
    return nc


def _get_nc():
    global _NC
    if _NC is None:
        _NC = _build()
    return _NC


def _prep_maps(x, Wq, bq, Wk, bk, Wv, bv, W1, b1, W2, b2, gamma, beta):
    bf = ml_dtypes.bfloat16
    f4 = np.float32

    def bc(v, n=D):
        return np.ascontiguousarray(
            np.broadcast_to(np.asarray(v, f4), (128, n)))

    xt0 = np.ascontiguousarray(np.asarray(x[0], f4).T).astype(bf)
    xt1 = np.ascontiguousarray(np.asarray(x[1], f4).T).astype(bf)
    lnpack = np.concatenate([bc(b2), bc(gamma), bc(beta)], axis=1)
    shared = {
        "xt0": xt0, "xt1": xt1,
        "w1": np.ascontiguousarray(np.asarray(W1, f4)).astype(bf),
        "w2": np.ascontiguousarray(np.asarray(W2, f4)).astype(bf),
        "lnpack": np.ascontiguousarray(lnpack),
    }
    Wqf, Wkf, Wvf = (np.asarray(w, f4) for w in (Wq, Wk, Wv))
    bqf, bkf, bvf = (np.asarray(v, f4) for v in (bq, bk, bv))
    b1f = np.asarray(b1, f4)
    eye = np.eye(128, dtype=np.float32)
    in_maps = []
    for c in range(N_CORES):
        sl = slice(c * 128, (c + 1) * 128)
        # wpack: wk(8) | wq(8) | wv(8) | eye | -eye  as [1024,128] blocks
        # reshaped to [128, 26*128]: block i lives at cols [i*128,(i+1)*128)
        # with partition p = input-dim p within the block.
        blocks = ([Wkf[k * 128:(k + 1) * 128, sl] for k in range(CT)]
                  + [Wqf[k * 128:(k + 1) * 128, sl] for k in range(CT)]
                  + [Wvf[k * 128:(k + 1) * 128, sl] for k in range(CT)]
                  + [eye, -eye])
        wpack = np.concatenate(blocks, axis=1).astype(bf)
        fpack = np.concatenate([
            bkf[sl].reshape(128, 1), bqf[sl].reshape(128, 1),
            bvf[sl].reshape(128, 1), b1f.reshape(FT, 128).T], axis=1)
        in_maps.append({
            **shared,
            "wpack": np.ascontiguousarray(wpack),
            "fpack": np.ascontiguousarray(fpack.astype(f4)),
        })
    return in_maps


def run_full(inputs, trace=False):
    nc = _get_nc()
    in_maps = _prep_maps(**inputs)
    res = run_bass_kernel_spmd(nc, in_maps, core_ids=list(range(N_CORES)),
                               trace=trace)
    y = np.empty((B, S, D), np.float32)
    for c in range(N_CORES):
        b, q0 = c // (N_CORES // B), (c % (N_CORES // B)) * QB
        y[b, q0:q0 + QB, :] = res.results[c]["y"]
    return y, res


def kernel(**inputs):
    y, _ = run_full(inputs, trace=False)
    return y
